# revision 28
# baseline (speedup 1.0000x reference)
"""Trainium2 Bass kernel for nn_Decoder_20701742366850.

Math notes (verified against the reference to fp32 precision):
  * softmax over a size-1 axis is identically 1.0, so the attention-score
    branch (Wa1/Wa2 and the pairwise rel embedding) never affects the output.
  * The pooled tensor wt[s,i,j,:] = 0.05*base[s,j,:] is independent of i, so
    the P x P pooling collapses to a per-row MLP followed by a max over each
    group, broadcast back to the group's rows.
  * Positions (last_pos/cur) and velocity embeddings are dead downstream.
  * max_j relu(x_j + b) = relu(max_j x_j + b): the group max is taken on the
    raw matmul accumulator and bias+relu applied to the (128, G) result.

What remains per step, per row: LSTM cell -> h2p -> (m1 -> m2 -> group max)
-> d1 -> d2, which is what the device kernel computes.

Sharding: data parallel over the 32 scenes; each of the 8 cores owns 4
contiguous groups = 128 rows. All weights replicated; no cross-core comms.
Layout on device is feature-major (features on partitions, rows on the free
axis), so per-feature biases are per-partition ACT biases and the group max
is a free-axis windowed reduction. The group-max broadcast back to rows is
done with a stride-0 rhs access pattern on the d1 matmuls (free).
"""

import numpy as np

try:
    import concourse.bass as bass  # noqa: F401
except Exception:  # pragma: no cover - fresh grading dir
    import sys
    for _p in ("/opt/trn_rl_repo",):
        if _p not in sys.path:
            sys.path.insert(0, _p)

import concourse.bacc as bacc
import concourse.bass as bass
import concourse.mybir as mybir
from concourse import tile
from concourse.tile_rust import add_dep_helper
from concourse.bass_utils import run_bass_kernel_spmd
import ml_dtypes

bf16 = ml_dtypes.bfloat16

SEQ = 12
E = 64
HD = 128
G = 32
P = 32
B = G * P
BOT = 1024
MLP = 1024
NCORES = 8
R = B // NCORES            # rows per core
GC = G // NCORES           # groups per core
H = R // 2                 # free-dim half for latency-split elementwise ops
BN_S = float(1.0 / np.sqrt(1.0 + 1e-5))

F32 = mybir.dt.float32
BF = mybir.dt.bfloat16

# matmul stream dtype: bf16 (fast) or fp32 (exact, ~2.5x slower)
MM_BF16 = True

AF = mybir.ActivationFunctionType
ALU = mybir.AluOpType


def _build_program(mm_bf16: bool):
    """Build the per-core Bass program."""
    MD = BF if mm_bf16 else F32

    # packed weight column maps; each chunk occupies [0:K, col:col+M]
    a_cols = 64 + 64 + 4 * 128 + 4 * 128 + 64 + 4 * 128 + 4 * 128   # atc,sp,ih,hh,disp,m1h,m1a
    b_cols = 32 * 128
    c_cols = 72 * 128
    d_cols = 8 * 128
    NB = 64

    nc = bacc.Bacc(None, target_bir_lowering=False)

    wpa_d = nc.dram_tensor("wpa", [128, a_cols], MD, kind="ExternalInput")
    wpb_d = nc.dram_tensor("wpb", [128, b_cols], MD, kind="ExternalInput")
    wpc_d = nc.dram_tensor("wpc", [128, c_cols], MD, kind="ExternalInput")
    wpd_d = nc.dram_tensor("wpd", [128, d_cols], MD, kind="ExternalInput")
    bia_d = nc.dram_tensor("bia", [128, NB], F32, kind="ExternalInput")
    st_d = nc.dram_tensor("st", [128, 3 * R], MD, kind="ExternalInput")
    c0_d = nc.dram_tensor("c0t", [128, R], F32, kind="ExternalInput")

    rels_d = nc.dram_tensor("rels", [SEQ, 2, R], F32, kind="ExternalOutput")
    hout_d = nc.dram_tensor("hout", [128, R], F32, kind="ExternalOutput")

    with tile.TileContext(nc) as tc:
        with (
            tc.tile_pool(name="wgt", bufs=1) as wgt,
            tc.tile_pool(name="state", bufs=1) as state,
            tc.tile_pool(name="act", bufs=2) as act,
            tc.tile_pool(name="ps", bufs=3, space="PSUM") as ps,
            tc.tile_pool(name="psg", bufs=2, space="PSUM") as psg,
            tc.tile_pool(name="pss", bufs=1, space="PSUM") as pss,
        ):
            # ---- initial state (packed; enqueued before the big weights) ----
            stt = state.tile([128, 3 * R], MD, tag="st0")
            cT = state.tile([128, R], F32, tag="c0")
            h_bf = stt[:, 0:R]
            di_bf = stt[0:64, R:2 * R]          # host-computed lpr@W_sp + b_sp
            at0 = stt[0:1, 2 * R:3 * R]

            def _state_dma_slot():
                nc.gpsimd.dma_start(stt[:], st_d[:])
                nc.gpsimd.dma_start(cT[:], c0_d[:])

            wpa = wgt.tile([128, a_cols], MD)
            wpb = wgt.tile([128, b_cols], MD)
            wpc = wgt.tile([128, c_cols], MD)
            wpd = wgt.tile([128, d_cols], MD)
            bia = wgt.tile([128, NB], F32)
            # order + queue spread matters: the prologue needs bia/wpa and the
            # tiny state tensors immediately; the big mid/late weights (wpb,
            # wpc, wpd) go on other queues so they don't block the start.
            # gpsimd's software-DGE queue is by far the fastest path
            # (~250 GB/s vs ~4 GB/s on the sync HWDGE queue for these 2D
            # patterns); everything startup-critical goes there, in need
            # order. wpb/wpd ride the scalar HWDGE queue in parallel.
            nc.gpsimd.dma_start(bia[:], bia_d[:])
            # placeholder: state DMAs enqueued here (before the big weights)
            _state_dma_slot()
            c0_ = 128                                  # atc+wsp
            c1_ = 128 + 4 * 128 + 4 * 128              # + ih + hh
            nc.gpsimd.dma_start(wpa[:, 0:c0_], wpa_d[:, 0:c0_])
            nc.gpsimd.dma_start(wpa[:, c0_:c1_], wpa_d[:, c0_:c1_])
            nc.gpsimd.dma_start(wpa[:, c1_:], wpa_d[:, c1_:])
            nc.scalar.dma_start(wpb[:], wpb_d[:])
            ch = 8 * 128 + 4 * 8 * 128                # d1h + first 4 k-rows of d1p
            nc.gpsimd.dma_start(wpc[:, 0:ch], wpc_d[:, 0:ch])
            nc.gpsimd.dma_start(wpc[:, ch:], wpc_d[:, ch:])
            nc.scalar.dma_start(wpd[:], wpd_d[:])

            off = [0]

            def _chunk(pool_tile, K, M):
                c = off[0]
                off[0] += M
                return pool_tile[0:K, c:c + M]

            atc = _chunk(wpa, 1, 64)
            wsp = _chunk(wpa, 2, 64)
            ih = [_chunk(wpa, 64, 128) for _ in range(4)]
            hh = [_chunk(wpa, 128, 128) for _ in range(4)]
            disp = _chunk(wpa, 128, 64)
            m1h = [_chunk(wpa, 128, 128) for _ in range(4)]
            m1a = [_chunk(wpa, 64, 128) for _ in range(4)]
            assert off[0] == a_cols
            m2w = [[wpb[:, (k * 8 + m) * 128:(k * 8 + m) * 128 + 128]
                    for m in range(8)] for k in range(4)]
            d1h = [wpc[:, m * 128:m * 128 + 128] for m in range(8)]
            d1p = [[wpc[:, (8 + k * 8 + m) * 128:(8 + k * 8 + m) * 128 + 128]
                    for m in range(8)] for k in range(8)]
            d2w = [wpd[:, k * 128:k * 128 + 128] for k in range(8)]

            bg = [bia[:, n:n + 1] for n in range(4)]          # lstm gates (i,f,g,o)
            bdisp = bia[0:64, 4:5]
            bm1 = [bia[:, 5 + n:6 + n] for n in range(4)]
            bm2 = [bia[:, 9 + m:10 + m] for m in range(8)]
            bd1 = [bia[:, 17 + m:18 + m] for m in range(8)]
            bd2 = bia[:, 25:26]
            bpat = bia[0:64, 26:27]
            bh2p = bia[0:2, 27:28]
            bsp = bia[0:64, 28:29]
            wh2p = bia[:, 29:31]
            bm2a = bia[:, 32:64]

            # at_emb (64, R)
            at_ps = pss.tile([64, R], F32, tag="sm")
            nc.tensor.matmul(at_ps[:], atc, at0, start=True, stop=True)
            atT = state.tile([64, R], MD, tag="atemb")
            nc.scalar.activation(atT[:], at_ps[:], AF.Identity, bias=bpat, scale=1.0)

            def gate_psums():
                # 4 gate accumulators packed into two banks: A = (i,f), B = (g,o)
                gA = psg.tile([128, 256], F32, tag="gA")
                gB = psg.tile([128, 256], F32, tag="gB")
                return [gA[:, 0:128], gA[:, 128:256], gB[:, 0:128], gB[:, 128:256]]

            def issue_ih(di_tile):
                # start=True only on the first matmul touching each bank: a
                # PSUM "start" zeroes the whole 2KB zero region (= bank).
                g = gate_psums()
                for n in range(4):
                    nc.tensor.matmul(g[n], ih[n],
                                     di_tile if isinstance(di_tile, bass.AP) else di_tile[:],
                                     start=(n % 2 == 0), stop=False)
                return g

            g_cur = issue_ih(di_bf)

            for t in range(SEQ):
                last = t == SEQ - 1
                # ---- finish gates: hh part ----
                for n in range(4):
                    nc.tensor.matmul(g_cur[n], hh[n], h_bf if isinstance(h_bf, bass.AP) else h_bf[:], start=False,
                                     stop=(n % 2 == 1))

                # ---- LSTM elementwise ----
                sig_i = act.tile([128, R], F32, tag="sig_i")
                sig_f = act.tile([128, R], F32, tag="sig_f")
                tnh_g = act.tile([128, R], F32, tag="tnh_g")
                sig_o = act.tile([128, R], F32, tag="sig_o")
                cN = state.tile([128, R], F32, tag=f"c{(t + 1) % 2}")
                tnh_c = act.tile([128, R], F32, tag="tnh_c")
                hl_bf = act.tile([128, R], MD, tag="hlbf")
                t1 = act.tile([128, R], F32, tag="t1")
                t2 = act.tile([128, R], F32, tag="t2")
                i_sigf_b = nc.scalar.activation(sig_f[:], g_cur[1], AF.Sigmoid, bias=bg[1], scale=1.0)
                nc.scalar.activation(sig_i[:], g_cur[0], AF.Sigmoid, bias=bg[0], scale=1.0)
                nc.scalar.activation(tnh_g[:], g_cur[2], AF.Tanh, bias=bg[2], scale=1.0)
                nc.scalar.activation(sig_o[:], g_cur[3], AF.Sigmoid, bias=bg[3], scale=1.0)
                i_sigf = i_sigf_b.ins
                nc.vector.tensor_mul(t1[:], sig_f[:], cT[:])
                nc.vector.tensor_mul(t2[:], sig_i[:], tnh_g[:])
                nc.vector.tensor_add(cN[:], t1[:], t2[:])
                nc.scalar.activation(tnh_c[:], cN[:], AF.Tanh, bias=0.0, scale=1.0)
                nc.vector.tensor_mul(hl_bf[:], sig_o[:], tnh_c[:])
                cT = cN

                # ---- PE heat through the LSTM-chain window: real matmuls
                # pinned (via artificial dep on sig_f) to execute inside the
                # serial elementwise stretch so HAM never re-throttles. ----
                m1_ps = []
                for n in range(4):
                    mp = ps.tile([128, R], F32, tag="ps")
                    mm = nc.tensor.matmul(mp[:], m1a[n], atT[:], start=True, stop=False)
                    add_dep_helper(mm.ins, i_sigf, reason="boundary-gap filler")
                    m1_ps.append(mp)
                scr = pss.tile([128, R], F32, tag="sm")
                for n in range(30):
                    mm = nc.tensor.matmul(scr[:], hh[n % 4],
                                          h_bf if isinstance(h_bf, bass.AP) else h_bf[:],
                                          start=True, stop=True)
                    add_dep_helper(mm.ins, i_sigf, reason="boundary-gap heater")
                i_hl = nc.vector.tensor_mul(hl_bf[0:1, 0:1], sig_o[0:1, 0:1],
                                            tnh_c[0:1, 0:1])

                # ---- m1 h-part ----
                for n in range(4):
                    nc.tensor.matmul(m1_ps[n][:], m1h[n], hl_bf[:],
                                     start=False, stop=True)

                # fp32 hl (only for the h2p output head, off the critical path)
                hlF = act.tile([128, R], F32, tag="hlF")
                nc.vector.tensor_mul(hlF[:], sig_o[:], tnh_c[:])
                rel_ps = pss.tile([2, R], F32, tag="sm")
                nc.tensor.matmul(rel_ps[:], wh2p, hlF[:], start=True, stop=True)
                relT = act.tile([2, R], F32, tag="rel")
                nc.scalar.activation(relT[:], rel_ps[:], AF.Identity, bias=bh2p, scale=1.0)
                nc.gpsimd.dma_start(rels_d[t], relT[:])

                if not last:
                    di_ps = pss.tile([64, R], F32, tag="sm")
                    nc.tensor.matmul(di_ps[:], disp, hl_bf[:], start=True, stop=True)
                    di_bf = act.tile([64, R], MD, tag="di")
                    nc.scalar.activation(di_bf[:], di_ps[:], AF.Identity, bias=bdisp, scale=1.0)

                # ---- m1 relu (split across ACT and DVE for latency) ----
                m1_bf = []
                for n in range(4):
                    mt = act.tile([128, R], MD, tag=f"m1s_{n}")
                    if n % 2 == 0:
                        nc.scalar.activation(mt[:], m1_ps[n][:], AF.Relu, bias=bm1[n], scale=1.0)
                    else:
                        nc.vector.tensor_scalar(mt[:], m1_ps[n][:], bm1[n], 0.0,
                                                op0=ALU.add, op1=ALU.max)
                    m1_bf.append(mt)
                for n in range(4):
                    mm = nc.tensor.matmul(scr[:], hh[n % 4],
                                          h_bf if isinstance(h_bf, bass.AP) else h_bf[:],
                                          start=True, stop=True)
                    add_dep_helper(mm.ins, i_hl.ins, reason="m1-relu-wait heater")

                # ---- m2 + fused group-max (max before bias+relu) ----
                # two output chunks share one PSUM bank -> one zero-region
                # start per pair instead of per chunk
                phr = act.tile([128, 8 * GC], F32, tag="phr")
                for mp_i in range(4):
                    mp = ps.tile([128, 2 * R], F32, tag="ps")
                    for half in range(2):
                        m = 2 * mp_i + half
                        sl = mp[:, half * R:(half + 1) * R]
                        for k in range(4):
                            nc.tensor.matmul(sl, m2w[k][m], m1_bf[k][:],
                                             start=(half == 0 and k == 0),
                                             stop=(half == 1 and k == 3))
                    for half in range(2):
                        m = 2 * mp_i + half
                        sl = mp[:, half * R:(half + 1) * R]
                        nc.vector.reduce_max(phr[:, m * GC:(m + 1) * GC],
                                             sl.rearrange("p (g j) -> p g j", j=P),
                                             axis=mybir.AxisListType.X)
                # one fused bias+relu over all 8 chunk maxima
                phb = act.tile([128, 8 * GC], MD, tag="phb")
                nc.vector.tensor_tensor(phb[:], phr[:], bm2a[:],
                                        op=ALU.add)
                nc.vector.tensor_scalar(phb[:], phb[:], 0.0, None, op0=ALU.max)
                ph = [phb[:, m * GC:(m + 1) * GC] for m in range(8)]

                # ---- d1 (rotated ph order; two chunks per PSUM bank) ----
                d1_bf = [None] * 8
                for dp_i in range(4):
                    dp = ps.tile([128, 2 * R], F32, tag="ps")
                    for half in range(2):
                        m = 2 * dp_i + half
                        sl = dp[:, half * R:(half + 1) * R]
                        nc.tensor.matmul(sl, d1h[m], hl_bf[:],
                                         start=(half == 0), stop=False)
                        for j in range(8):
                            k = (m + j) % 8
                            nc.tensor.matmul(sl, d1p[k][m],
                                             ph[k].broadcast_to((128, GC, P)),
                                             start=False,
                                             stop=(half == 1 and j == 7))
                    for half in range(2):
                        m = 2 * dp_i + half
                        sl = dp[:, half * R:(half + 1) * R]
                        dt_ = act.tile([128, R], MD, tag=f"d1s_{m % 4}")
                        if half == 0:
                            nc.scalar.activation(dt_[:], sl, AF.Relu, bias=bd1[m], scale=1.0)
                        else:
                            nc.vector.tensor_scalar(dt_[:], sl, bd1[m], 0.0,
                                                    op0=ALU.add, op1=ALU.max)
                        d1_bf[m] = dt_

                # ---- d2 (accumulate in d1 completion order: 1,2,...,7,0) ----
                d2_ps = ps.tile([128, R], F32, tag="ps")
                order = [0, 1, 2, 3, 4, 5, 6, 7]
                for j, m in enumerate(order):
                    nc.tensor.matmul(d2_ps[:], d2w[m], d1_bf[m][:],
                                     start=(j == 0), stop=(j == 7))
                if not last:
                    h_bf = state.tile([128, R], MD, tag=f"hcarry{(t + 1) % 2}")
                    # DVE is idle here and ~100ns faster than ACT for this
                    nc.vector.tensor_scalar(h_bf[:], d2_ps[:], bd2, 0.0,
                                            op0=ALU.add, op1=ALU.max)
                    # issue next step's input-gate matmuls late so the
                    # scheduler uses them as PE filler during the boundary
                    g_cur = issue_ih(di_bf)
                else:
                    h_f32 = state.tile([128, R], F32, tag="hfin")
                    nc.scalar.activation(h_f32[:], d2_ps[:], AF.Relu, bias=bd2, scale=1.0)
                    nc.gpsimd.dma_start(hout_d[:], h_f32[:])

    nc.compile()
    return nc, None


def _pack_weights(ins, mm_bf16: bool):
    """Host-side weight packing into the per-core packed arrays."""
    MDnp = bf16 if mm_bf16 else np.float32
    f32 = np.float32

    W_ih = np.asarray(ins["W_ih"], f32)
    W_hh = np.asarray(ins["W_hh"], f32)
    W_sp = np.asarray(ins["W_sp"], f32)
    W_h2p = np.asarray(ins["W_h2p"], f32)
    Wm1 = np.asarray(ins["Wm1"], f32) * (0.05 * BN_S)
    Wm2 = np.asarray(ins["Wm2"], f32) * BN_S
    Wd1 = np.asarray(ins["Wd1"], f32) * BN_S
    Wd2 = np.asarray(ins["Wd2"], f32) * BN_S
    Wp_at = np.asarray(ins["Wp_at"], f32)

    a_chunks = [(1, Wp_at.sum(0, keepdims=True)),             # atc (1,64)
                (2, W_sp)]                                    # wsp (2,64)
    for n in range(4):
        a_chunks.append((64, W_ih[:, n * 128:(n + 1) * 128]))
    for n in range(4):
        a_chunks.append((128, W_hh[:, n * 128:(n + 1) * 128]))
    a_chunks.append((128, W_h2p @ W_sp))                      # disp (128,64)
    for n in range(4):
        a_chunks.append((128, Wm1[0:128, n * 128:(n + 1) * 128]))
    for n in range(4):
        a_chunks.append((64, Wm1[128:192, n * 128:(n + 1) * 128]))

    def pack(chunks):
        ncols = sum(c.shape[1] for _, c in chunks)
        out = np.zeros((128, ncols), dtype=MDnp)
        col = 0
        for K, c in chunks:
            assert c.shape[0] == K
            out[0:K, col:col + c.shape[1]] = c.astype(MDnp)
            col += c.shape[1]
        return out

    wpa = pack(a_chunks)
    wpb = pack([(128, Wm2[k * 128:(k + 1) * 128, m * 128:(m + 1) * 128])
                for k in range(4) for m in range(8)])
    wpc = pack([(128, Wd1[0:128, m * 128:(m + 1) * 128]) for m in range(8)] +
               [(128, Wd1[128 + k * 128:128 + (k + 1) * 128, m * 128:(m + 1) * 128])
                for k in range(8) for m in range(8)])
    wpd = pack([(128, Wd2[k * 128:(k + 1) * 128, 0:128]) for k in range(8)])

    b_ih = np.asarray(ins["b_ih"], f32)
    b_hh = np.asarray(ins["b_hh"], f32)
    b_sp = np.asarray(ins["b_sp"], f32)
    b_h2p = np.asarray(ins["b_h2p"], f32)
    bm1 = np.asarray(ins["bm1"], f32) * BN_S
    bm2 = np.asarray(ins["bm2"], f32) * BN_S
    bd1 = np.asarray(ins["bd1"], f32) * BN_S
    bd2 = np.asarray(ins["bd2"], f32) * BN_S
    bp_at = np.asarray(ins["bp_at"], f32)
    bg = b_ih + b_hh

    bia = np.zeros((128, 64), dtype=f32)
    for n in range(4):
        bia[:, n] = bg[n * 128:(n + 1) * 128]
    bia[0:64, 4] = b_h2p @ W_sp + b_sp                        # bdisp
    for n in range(4):
        bia[:, 5 + n] = bm1[n * 128:(n + 1) * 128]
    for m in range(8):
        bia[:, 9 + m] = bm2[m * 128:(m + 1) * 128]
    for m in range(8):
        bia[:, 17 + m] = bd1[m * 128:(m + 1) * 128]
    bia[:, 25] = bd2
    bia[0:64, 26] = bp_at
    bia[0:2, 27] = b_h2p
    bia[0:64, 28] = b_sp
    bia[:, 29:31] = W_h2p
    for m in range(8):
        for g in range(4):
            bia[:, 32 + m * 4 + g] = bm2[m * 128:(m + 1) * 128]

    return dict(wpa=wpa, wpb=wpb, wpc=wpc, wpd=wpd, bia=bia)


_CACHE = {}


def _get_program(mm_bf16: bool):
    if mm_bf16 not in _CACHE:
        _CACHE[mm_bf16] = _build_program(mm_bf16)
    return _CACHE[mm_bf16]


def run(inputs, mm_bf16=MM_BF16, trace=False, **spmd_kwargs):
    ins = {k: np.asarray(v) for k, v in inputs.items()}
    nc, _ = _get_program(mm_bf16)
    wmap = _pack_weights(ins, mm_bf16)

    MDnp = bf16 if mm_bf16 else np.float32
    h0 = ins["h0"][0].astype(np.float32)        # (B,HD)
    c0 = ins["c0"][0].astype(np.float32)
    at0 = ins["agent_type"][0, :, 0].astype(np.float32)   # (B,)
    lpr = ins["last_pos_rel"].astype(np.float32)          # (B,2)
    W_sp = np.asarray(ins["W_sp"], np.float32)
    b_sp = np.asarray(ins["b_sp"], np.float32)

    in_maps = []
    for k in range(NCORES):
        rows = slice(k * R, (k + 1) * R)
        m = dict(wmap)
        st = np.zeros((128, 3 * R), dtype=MDnp)
        st[:, 0:R] = h0[rows].T.astype(MDnp)
        di0 = lpr[rows] @ W_sp + b_sp                     # (R, 64) fp32
        st[0:64, R:2 * R] = di0.T.astype(MDnp)
        st[0, 2 * R:3 * R] = at0[rows].astype(MDnp)
        m["st"] = st
        m["c0t"] = np.ascontiguousarray(c0[rows].T)
        in_maps.append(m)

    res = run_bass_kernel_spmd(nc, in_maps, list(range(NCORES)),
                               trace=trace, **spmd_kwargs)

    rels = np.empty((SEQ, B, 2), dtype=np.float32)
    hout = np.empty((1, B, HD), dtype=np.float32)
    for k in range(NCORES):
        rows = slice(k * R, (k + 1) * R)
        rk = res.results[k]["rels"]            # (SEQ,2,R)
        rels[:, rows, :] = np.transpose(rk, (0, 2, 1))
        hout[0, rows, :] = res.results[k]["hout"].T
    return (rels, hout), res


def kernel(**inputs):
    (rels, hout), _ = run(inputs)
    return rels, hout


# revision 29
# speedup vs baseline: 1.0443x; 1.0443x over previous
"""Trainium2 Bass kernel for nn_Decoder_20701742366850.

Math notes (verified against the reference to fp32 precision):
  * softmax over a size-1 axis is identically 1.0, so the attention-score
    branch (Wa1/Wa2 and the pairwise rel embedding) never affects the output.
  * The pooled tensor wt[s,i,j,:] = 0.05*base[s,j,:] is independent of i, so
    the P x P pooling collapses to a per-row MLP followed by a max over each
    group, broadcast back to the group's rows.
  * Positions (last_pos/cur) and velocity embeddings are dead downstream.
  * max_j relu(x_j + b) = relu(max_j x_j + b): the group max is taken on the
    raw matmul accumulator and bias+relu applied to the (128, G) result.

What remains per step, per row: LSTM cell -> h2p -> (m1 -> m2 -> group max)
-> d1 -> d2, which is what the device kernel computes.

Sharding: data parallel over the 32 scenes; each of the 8 cores owns 4
contiguous groups = 128 rows. All weights replicated; no cross-core comms.
Layout on device is feature-major (features on partitions, rows on the free
axis), so per-feature biases are per-partition ACT biases and the group max
is a free-axis windowed reduction. The group-max broadcast back to rows is
done with a stride-0 rhs access pattern on the d1 matmuls (free).
"""

import numpy as np

try:
    import concourse.bass as bass  # noqa: F401
except Exception:  # pragma: no cover - fresh grading dir
    import sys
    for _p in ("/opt/trn_rl_repo",):
        if _p not in sys.path:
            sys.path.insert(0, _p)

import concourse.bacc as bacc
import concourse.bass as bass
import concourse.mybir as mybir
from concourse import tile
from concourse.tile_rust import add_dep_helper
from concourse.bass_utils import run_bass_kernel_spmd
import ml_dtypes

bf16 = ml_dtypes.bfloat16

SEQ = 12
E = 64
HD = 128
G = 32
P = 32
B = G * P
BOT = 1024
MLP = 1024
NCORES = 8
R = B // NCORES            # rows per core
GC = G // NCORES           # groups per core
H = R // 2                 # free-dim half for latency-split elementwise ops
BN_S = float(1.0 / np.sqrt(1.0 + 1e-5))

F32 = mybir.dt.float32
BF = mybir.dt.bfloat16

# matmul stream dtype: bf16 (fast) or fp32 (exact, ~2.5x slower)
MM_BF16 = True

AF = mybir.ActivationFunctionType
ALU = mybir.AluOpType


def _build_program(mm_bf16: bool):
    """Build the per-core Bass program."""
    MD = BF if mm_bf16 else F32

    # packed weight column maps; each chunk occupies [0:K, col:col+M]
    a_cols = 64 + 64 + 4 * 128 + 4 * 128 + 64 + 4 * 128 + 4 * 128   # atc,sp,ih,hh,disp,m1h,m1a
    b_cols = 32 * 128
    c_cols = 72 * 128
    d_cols = 8 * 128
    NB = 64

    nc = bacc.Bacc(None, target_bir_lowering=False)

    wpa_d = nc.dram_tensor("wpa", [128, a_cols], MD, kind="ExternalInput")
    wpb_d = nc.dram_tensor("wpb", [128, b_cols], MD, kind="ExternalInput")
    wpc_d = nc.dram_tensor("wpc", [128, c_cols], MD, kind="ExternalInput")
    wpd_d = nc.dram_tensor("wpd", [128, d_cols], MD, kind="ExternalInput")
    bia_d = nc.dram_tensor("bia", [128, NB], F32, kind="ExternalInput")
    st_d = nc.dram_tensor("st", [128, 3 * R], MD, kind="ExternalInput")
    c0_d = nc.dram_tensor("c0t", [128, R], F32, kind="ExternalInput")

    rels_d = nc.dram_tensor("rels", [SEQ, 2, R], F32, kind="ExternalOutput")
    hout_d = nc.dram_tensor("hout", [128, R], F32, kind="ExternalOutput")

    with tile.TileContext(nc) as tc:
        with (
            tc.tile_pool(name="wgt", bufs=1) as wgt,
            tc.tile_pool(name="state", bufs=1) as state,
            tc.tile_pool(name="act", bufs=2) as act,
            tc.tile_pool(name="ps", bufs=3, space="PSUM") as ps,
            tc.tile_pool(name="psg", bufs=2, space="PSUM") as psg,
            tc.tile_pool(name="pss", bufs=1, space="PSUM") as pss,
        ):
            # ---- initial state (packed; enqueued before the big weights) ----
            stt = state.tile([128, 3 * R], MD, tag="st0")
            cT = state.tile([128, R], F32, tag="c0")
            h_bf = stt[:, 0:R]
            di_bf = stt[0:64, R:2 * R]          # host-computed lpr@W_sp + b_sp
            at0 = stt[0:1, 2 * R:3 * R]

            def _state_dma_slot():
                nc.gpsimd.dma_start(stt[:], st_d[:])
                nc.gpsimd.dma_start(cT[:], c0_d[:])

            wpa = wgt.tile([128, a_cols], MD)
            wpb = wgt.tile([128, b_cols], MD)
            wpc = wgt.tile([128, c_cols], MD)
            wpd = wgt.tile([128, d_cols], MD)
            bia = wgt.tile([128, NB], F32)
            # order + queue spread matters: the prologue needs bia/wpa and the
            # tiny state tensors immediately; the big mid/late weights (wpb,
            # wpc, wpd) go on other queues so they don't block the start.
            # gpsimd's software-DGE queue is by far the fastest path
            # (~250 GB/s vs ~4 GB/s on the sync HWDGE queue for these 2D
            # patterns); everything startup-critical goes there, in need
            # order. wpb/wpd ride the scalar HWDGE queue in parallel.
            nc.gpsimd.dma_start(bia[:], bia_d[:])
            # placeholder: state DMAs enqueued here (before the big weights)
            _state_dma_slot()
            c0_ = 128                                  # atc+wsp
            c1_ = 128 + 4 * 128 + 4 * 128              # + ih + hh
            nc.gpsimd.dma_start(wpa[:, 0:c0_], wpa_d[:, 0:c0_])
            nc.gpsimd.dma_start(wpa[:, c0_:c1_], wpa_d[:, c0_:c1_])
            nc.gpsimd.dma_start(wpa[:, c1_:], wpa_d[:, c1_:])
            nc.scalar.dma_start(wpb[:], wpb_d[:])
            ch = 8 * 128 + 4 * 8 * 128                # d1h + first 4 k-rows of d1p
            nc.gpsimd.dma_start(wpc[:, 0:ch], wpc_d[:, 0:ch])
            nc.gpsimd.dma_start(wpc[:, ch:], wpc_d[:, ch:])
            nc.scalar.dma_start(wpd[:], wpd_d[:])

            off = [0]

            def _chunk(pool_tile, K, M):
                c = off[0]
                off[0] += M
                return pool_tile[0:K, c:c + M]

            atc = _chunk(wpa, 1, 64)
            wsp = _chunk(wpa, 2, 64)
            ih = [_chunk(wpa, 64, 128) for _ in range(4)]
            hh = [_chunk(wpa, 128, 128) for _ in range(4)]
            disp = _chunk(wpa, 128, 64)
            m1h = [_chunk(wpa, 128, 128) for _ in range(4)]
            m1a = [_chunk(wpa, 64, 128) for _ in range(4)]
            assert off[0] == a_cols
            m2w = [[wpb[:, (k * 8 + m) * 128:(k * 8 + m) * 128 + 128]
                    for m in range(8)] for k in range(4)]
            d1h = [wpc[:, m * 128:m * 128 + 128] for m in range(8)]
            d1p = [[wpc[:, (8 + k * 8 + m) * 128:(8 + k * 8 + m) * 128 + 128]
                    for m in range(8)] for k in range(8)]
            d2w = [wpd[:, k * 128:k * 128 + 128] for k in range(8)]

            bg = [bia[:, n:n + 1] for n in range(4)]          # lstm gates (i,f,g,o)
            bdisp = bia[0:64, 4:5]
            bm1 = [bia[:, 5 + n:6 + n] for n in range(4)]
            bm2 = [bia[:, 9 + m:10 + m] for m in range(8)]
            bd1 = [bia[:, 17 + m:18 + m] for m in range(8)]
            bd2 = bia[:, 25:26]
            bpat = bia[0:64, 26:27]
            bh2p = bia[0:2, 27:28]
            bsp = bia[0:64, 28:29]
            wh2p = bia[:, 29:31]
            bm2a = bia[:, 32:64]

            # at_emb (64, R)
            at_ps = pss.tile([64, R], F32, tag="sm")
            nc.tensor.matmul(at_ps[:], atc, at0, start=True, stop=True)
            atT = state.tile([64, R], MD, tag="atemb")
            nc.scalar.activation(atT[:], at_ps[:], AF.Identity, bias=bpat, scale=1.0)

            def gate_psums():
                # 4 gate accumulators packed into two banks: A = (i,f), B = (g,o)
                gA = psg.tile([128, 256], F32, tag="gA")
                gB = psg.tile([128, 256], F32, tag="gB")
                return [gA[:, 0:128], gA[:, 128:256], gB[:, 0:128], gB[:, 128:256]]

            def issue_ih(di_tile):
                # start=True only on the first matmul touching each bank: a
                # PSUM "start" zeroes the whole 2KB zero region (= bank).
                g = gate_psums()
                for n in range(4):
                    nc.tensor.matmul(g[n], ih[n],
                                     di_tile if isinstance(di_tile, bass.AP) else di_tile[:],
                                     start=(n % 2 == 0), stop=False)
                return g

            g_cur = issue_ih(di_bf)

            for t in range(SEQ):
                last = t == SEQ - 1
                # ---- finish gates: hh part ----
                for n in range(4):
                    nc.tensor.matmul(g_cur[n], hh[n], h_bf if isinstance(h_bf, bass.AP) else h_bf[:], start=False,
                                     stop=(n % 2 == 1))

                # ---- LSTM elementwise ----
                sig_i = act.tile([128, R], F32, tag="sig_i")
                sig_f = act.tile([128, R], F32, tag="sig_f")
                tnh_g = act.tile([128, R], F32, tag="tnh_g")
                sig_o = act.tile([128, R], F32, tag="sig_o")
                cN = state.tile([128, R], F32, tag=f"c{(t + 1) % 2}")
                tnh_c = act.tile([128, R], F32, tag="tnh_c")
                hl_bf = act.tile([128, R], MD, tag="hlbf")
                t1 = act.tile([128, R], F32, tag="t1")
                t2 = act.tile([128, R], F32, tag="t2")
                i_sigf_b = nc.scalar.activation(sig_f[:], g_cur[1], AF.Sigmoid, bias=bg[1], scale=1.0)
                nc.scalar.activation(sig_i[:], g_cur[0], AF.Sigmoid, bias=bg[0], scale=1.0)
                nc.scalar.activation(tnh_g[:], g_cur[2], AF.Tanh, bias=bg[2], scale=1.0)
                nc.scalar.activation(sig_o[:], g_cur[3], AF.Sigmoid, bias=bg[3], scale=1.0)
                i_sigf = i_sigf_b.ins
                nc.vector.tensor_mul(t1[:], sig_f[:], cT[:])
                nc.vector.tensor_mul(t2[:], sig_i[:], tnh_g[:])
                nc.vector.tensor_add(cN[:], t1[:], t2[:])
                nc.scalar.activation(tnh_c[:], cN[:], AF.Tanh, bias=0.0, scale=1.0)
                nc.vector.tensor_mul(hl_bf[:], sig_o[:], tnh_c[:])
                cT = cN

                # ---- PE heat through the LSTM-chain window: real matmuls
                # pinned (via artificial dep on sig_f) to execute inside the
                # serial elementwise stretch so HAM never re-throttles. ----
                m1_ps = []
                for n in range(4):
                    mp = ps.tile([128, R], F32, tag="ps")
                    mm = nc.tensor.matmul(mp[:], m1a[n], atT[:], start=True, stop=False)
                    add_dep_helper(mm.ins, i_sigf, reason="boundary-gap filler")
                    m1_ps.append(mp)
                scr = pss.tile([128, R], F32, tag="sm")
                for n in range(30):
                    mm = nc.tensor.matmul(scr[:], hh[n % 4],
                                          h_bf if isinstance(h_bf, bass.AP) else h_bf[:],
                                          start=True, stop=True)
                    add_dep_helper(mm.ins, i_sigf, reason="boundary-gap heater")
                i_hl = nc.vector.tensor_mul(hl_bf[0:1, 0:1], sig_o[0:1, 0:1],
                                            tnh_c[0:1, 0:1])

                # ---- m1 h-part ----
                for n in range(4):
                    nc.tensor.matmul(m1_ps[n][:], m1h[n], hl_bf[:],
                                     start=False, stop=True)

                # fp32 hl (only for the h2p output head, off the critical path)
                hlF = act.tile([128, R], F32, tag="hlF")
                nc.vector.tensor_mul(hlF[:], sig_o[:], tnh_c[:])
                rel_ps = pss.tile([2, R], F32, tag="sm")
                nc.tensor.matmul(rel_ps[:], wh2p, hlF[:], start=True, stop=True)
                relT = act.tile([2, R], F32, tag="rel")
                nc.scalar.activation(relT[:], rel_ps[:], AF.Identity, bias=bh2p, scale=1.0)
                nc.gpsimd.dma_start(rels_d[t], relT[:])

                if not last:
                    di_ps = pss.tile([64, R], F32, tag="sm")
                    nc.tensor.matmul(di_ps[:], disp, hl_bf[:], start=True, stop=True)
                    di_bf = act.tile([64, R], MD, tag="di")
                    nc.scalar.activation(di_bf[:], di_ps[:], AF.Identity, bias=bdisp, scale=1.0)

                # ---- m1 relu (split across ACT and DVE for latency) ----
                m1_bf = []
                for n in range(4):
                    mt = act.tile([128, R], MD, tag=f"m1s_{n}")
                    if n % 2 == 0:
                        nc.scalar.activation(mt[:], m1_ps[n][:], AF.Relu, bias=bm1[n], scale=1.0)
                    else:
                        nc.vector.tensor_scalar(mt[:], m1_ps[n][:], bm1[n], 0.0,
                                                op0=ALU.add, op1=ALU.max)
                    m1_bf.append(mt)
                for n in range(4):
                    mm = nc.tensor.matmul(scr[:], hh[n % 4],
                                          h_bf if isinstance(h_bf, bass.AP) else h_bf[:],
                                          start=True, stop=True)
                    add_dep_helper(mm.ins, i_hl.ins, reason="m1-relu-wait heater")

                # ---- m2 + fused group-max (max before bias+relu) ----
                # two output chunks share one PSUM bank -> one zero-region
                # start per pair instead of per chunk
                ph = []
                for mp_i in range(4):
                    mp = ps.tile([128, 2 * R], F32, tag="ps")
                    for half in range(2):
                        m = 2 * mp_i + half
                        sl = mp[:, half * R:(half + 1) * R]
                        for k in range(4):
                            nc.tensor.matmul(sl, m2w[k][m], m1_bf[k][:],
                                             start=(half == 0 and k == 0),
                                             stop=(half == 1 and k == 3))
                    for half in range(2):
                        m = 2 * mp_i + half
                        sl = mp[:, half * R:(half + 1) * R]
                        pr = act.tile([128, GC], F32, tag=f"phr_{m}")
                        nc.vector.reduce_max(pr[:], sl.rearrange("p (g j) -> p g j", j=P),
                                             axis=mybir.AxisListType.X)
                        pt = act.tile([128, GC], MD, tag=f"ph_{m}")
                        nc.vector.tensor_scalar(pt[:], pr[:], bm2[m], 0.0,
                                                op0=ALU.add, op1=ALU.max)
                        ph.append(pt)

                # ---- d1 (rotated ph order; two chunks per PSUM bank) ----
                d1_bf = [None] * 8
                for dp_i in range(4):
                    dp = ps.tile([128, 2 * R], F32, tag="ps")
                    for half in range(2):
                        m = 2 * dp_i + half
                        sl = dp[:, half * R:(half + 1) * R]
                        nc.tensor.matmul(sl, d1h[m], hl_bf[:],
                                         start=(half == 0), stop=False)
                        for j in range(8):
                            k = (m + j) % 8
                            nc.tensor.matmul(sl, d1p[k][m],
                                             ph[k][:].broadcast_to((128, GC, P)),
                                             start=False,
                                             stop=(half == 1 and j == 7))
                    for half in range(2):
                        m = 2 * dp_i + half
                        sl = dp[:, half * R:(half + 1) * R]
                        dt_ = act.tile([128, R], MD, tag=f"d1s_{m % 4}")
                        if half == 0:
                            nc.scalar.activation(dt_[:], sl, AF.Relu, bias=bd1[m], scale=1.0)
                        else:
                            nc.vector.tensor_scalar(dt_[:], sl, bd1[m], 0.0,
                                                    op0=ALU.add, op1=ALU.max)
                        d1_bf[m] = dt_

                # ---- d2 (accumulate in d1 completion order: 1,2,...,7,0) ----
                d2_ps = ps.tile([128, R], F32, tag="ps")
                order = [0, 1, 2, 3, 4, 5, 6, 7]
                for j, m in enumerate(order):
                    nc.tensor.matmul(d2_ps[:], d2w[m], d1_bf[m][:],
                                     start=(j == 0), stop=(j == 7))
                if not last:
                    h_bf = state.tile([128, R], MD, tag=f"hcarry{(t + 1) % 2}")
                    # DVE is idle here and ~100ns faster than ACT for this
                    nc.vector.tensor_scalar(h_bf[:], d2_ps[:], bd2, 0.0,
                                            op0=ALU.add, op1=ALU.max)
                    # issue next step's input-gate matmuls late so the
                    # scheduler uses them as PE filler during the boundary
                    g_cur = issue_ih(di_bf)
                else:
                    h_f32 = state.tile([128, R], F32, tag="hfin")
                    nc.scalar.activation(h_f32[:], d2_ps[:], AF.Relu, bias=bd2, scale=1.0)
                    nc.gpsimd.dma_start(hout_d[:], h_f32[:])

    nc.compile()
    return nc, None


def _pack_weights(ins, mm_bf16: bool):
    """Host-side weight packing into the per-core packed arrays."""
    MDnp = bf16 if mm_bf16 else np.float32
    f32 = np.float32

    W_ih = np.asarray(ins["W_ih"], f32)
    W_hh = np.asarray(ins["W_hh"], f32)
    W_sp = np.asarray(ins["W_sp"], f32)
    W_h2p = np.asarray(ins["W_h2p"], f32)
    Wm1 = np.asarray(ins["Wm1"], f32) * (0.05 * BN_S)
    Wm2 = np.asarray(ins["Wm2"], f32) * BN_S
    Wd1 = np.asarray(ins["Wd1"], f32) * BN_S
    Wd2 = np.asarray(ins["Wd2"], f32) * BN_S
    Wp_at = np.asarray(ins["Wp_at"], f32)

    a_chunks = [(1, Wp_at.sum(0, keepdims=True)),             # atc (1,64)
                (2, W_sp)]                                    # wsp (2,64)
    for n in range(4):
        a_chunks.append((64, W_ih[:, n * 128:(n + 1) * 128]))
    for n in range(4):
        a_chunks.append((128, W_hh[:, n * 128:(n + 1) * 128]))
    a_chunks.append((128, W_h2p @ W_sp))                      # disp (128,64)
    for n in range(4):
        a_chunks.append((128, Wm1[0:128, n * 128:(n + 1) * 128]))
    for n in range(4):
        a_chunks.append((64, Wm1[128:192, n * 128:(n + 1) * 128]))

    def pack(chunks):
        ncols = sum(c.shape[1] for _, c in chunks)
        out = np.zeros((128, ncols), dtype=MDnp)
        col = 0
        for K, c in chunks:
            assert c.shape[0] == K
            out[0:K, col:col + c.shape[1]] = c.astype(MDnp)
            col += c.shape[1]
        return out

    wpa = pack(a_chunks)
    wpb = pack([(128, Wm2[k * 128:(k + 1) * 128, m * 128:(m + 1) * 128])
                for k in range(4) for m in range(8)])
    wpc = pack([(128, Wd1[0:128, m * 128:(m + 1) * 128]) for m in range(8)] +
               [(128, Wd1[128 + k * 128:128 + (k + 1) * 128, m * 128:(m + 1) * 128])
                for k in range(8) for m in range(8)])
    wpd = pack([(128, Wd2[k * 128:(k + 1) * 128, 0:128]) for k in range(8)])

    b_ih = np.asarray(ins["b_ih"], f32)
    b_hh = np.asarray(ins["b_hh"], f32)
    b_sp = np.asarray(ins["b_sp"], f32)
    b_h2p = np.asarray(ins["b_h2p"], f32)
    bm1 = np.asarray(ins["bm1"], f32) * BN_S
    bm2 = np.asarray(ins["bm2"], f32) * BN_S
    bd1 = np.asarray(ins["bd1"], f32) * BN_S
    bd2 = np.asarray(ins["bd2"], f32) * BN_S
    bp_at = np.asarray(ins["bp_at"], f32)
    bg = b_ih + b_hh

    bia = np.zeros((128, 64), dtype=f32)
    for n in range(4):
        bia[:, n] = bg[n * 128:(n + 1) * 128]
    bia[0:64, 4] = b_h2p @ W_sp + b_sp                        # bdisp
    for n in range(4):
        bia[:, 5 + n] = bm1[n * 128:(n + 1) * 128]
    for m in range(8):
        bia[:, 9 + m] = bm2[m * 128:(m + 1) * 128]
    for m in range(8):
        bia[:, 17 + m] = bd1[m * 128:(m + 1) * 128]
    bia[:, 25] = bd2
    bia[0:64, 26] = bp_at
    bia[0:2, 27] = b_h2p
    bia[0:64, 28] = b_sp
    bia[:, 29:31] = W_h2p
    for m in range(8):
        for g in range(4):
            bia[:, 32 + m * 4 + g] = bm2[m * 128:(m + 1) * 128]

    return dict(wpa=wpa, wpb=wpb, wpc=wpc, wpd=wpd, bia=bia)


_CACHE = {}


def _get_program(mm_bf16: bool):
    if mm_bf16 not in _CACHE:
        _CACHE[mm_bf16] = _build_program(mm_bf16)
    return _CACHE[mm_bf16]


def run(inputs, mm_bf16=MM_BF16, trace=False, **spmd_kwargs):
    ins = {k: np.asarray(v) for k, v in inputs.items()}
    nc, _ = _get_program(mm_bf16)
    wmap = _pack_weights(ins, mm_bf16)

    MDnp = bf16 if mm_bf16 else np.float32
    h0 = ins["h0"][0].astype(np.float32)        # (B,HD)
    c0 = ins["c0"][0].astype(np.float32)
    at0 = ins["agent_type"][0, :, 0].astype(np.float32)   # (B,)
    lpr = ins["last_pos_rel"].astype(np.float32)          # (B,2)
    W_sp = np.asarray(ins["W_sp"], np.float32)
    b_sp = np.asarray(ins["b_sp"], np.float32)

    in_maps = []
    for k in range(NCORES):
        rows = slice(k * R, (k + 1) * R)
        m = dict(wmap)
        st = np.zeros((128, 3 * R), dtype=MDnp)
        st[:, 0:R] = h0[rows].T.astype(MDnp)
        di0 = lpr[rows] @ W_sp + b_sp                     # (R, 64) fp32
        st[0:64, R:2 * R] = di0.T.astype(MDnp)
        st[0, 2 * R:3 * R] = at0[rows].astype(MDnp)
        m["st"] = st
        m["c0t"] = np.ascontiguousarray(c0[rows].T)
        in_maps.append(m)

    res = run_bass_kernel_spmd(nc, in_maps, list(range(NCORES)),
                               trace=trace, **spmd_kwargs)

    rels = np.empty((SEQ, B, 2), dtype=np.float32)
    hout = np.empty((1, B, HD), dtype=np.float32)
    for k in range(NCORES):
        rows = slice(k * R, (k + 1) * R)
        rk = res.results[k]["rels"]            # (SEQ,2,R)
        rels[:, rows, :] = np.transpose(rk, (0, 2, 1))
        hout[0, rows, :] = res.results[k]["hout"].T
    return (rels, hout), res


def kernel(**inputs):
    (rels, hout), _ = run(inputs)
    return rels, hout


# revision 30
# speedup vs baseline: 1.0569x; 1.0121x over previous
"""Trainium2 Bass kernel for nn_Decoder_20701742366850.

Math notes (verified against the reference to fp32 precision):
  * softmax over a size-1 axis is identically 1.0, so the attention-score
    branch (Wa1/Wa2 and the pairwise rel embedding) never affects the output.
  * The pooled tensor wt[s,i,j,:] = 0.05*base[s,j,:] is independent of i, so
    the P x P pooling collapses to a per-row MLP followed by a max over each
    group, broadcast back to the group's rows.
  * Positions (last_pos/cur) and velocity embeddings are dead downstream.
  * max_j relu(x_j + b) = relu(max_j x_j + b): the group max is taken on the
    raw matmul accumulator and bias+relu applied to the (128, G) result.

What remains per step, per row: LSTM cell -> h2p -> (m1 -> m2 -> group max)
-> d1 -> d2, which is what the device kernel computes.

Sharding: data parallel over the 32 scenes; each of the 8 cores owns 4
contiguous groups = 128 rows. All weights replicated; no cross-core comms.
Layout on device is feature-major (features on partitions, rows on the free
axis), so per-feature biases are per-partition ACT biases and the group max
is a free-axis windowed reduction. The group-max broadcast back to rows is
done with a stride-0 rhs access pattern on the d1 matmuls (free).
"""

import numpy as np

try:
    import concourse.bass as bass  # noqa: F401
except Exception:  # pragma: no cover - fresh grading dir
    import sys
    for _p in ("/opt/trn_rl_repo",):
        if _p not in sys.path:
            sys.path.insert(0, _p)

import concourse.bacc as bacc
import concourse.bass as bass
import concourse.mybir as mybir
from concourse import tile
from concourse.tile_rust import add_dep_helper
from concourse.bass_utils import run_bass_kernel_spmd
import ml_dtypes

bf16 = ml_dtypes.bfloat16

SEQ = 12
E = 64
HD = 128
G = 32
P = 32
B = G * P
BOT = 1024
MLP = 1024
NCORES = 8
R = B // NCORES            # rows per core
GC = G // NCORES           # groups per core
H = R // 2                 # free-dim half for latency-split elementwise ops
BN_S = float(1.0 / np.sqrt(1.0 + 1e-5))

F32 = mybir.dt.float32
BF = mybir.dt.bfloat16

# matmul stream dtype: bf16 (fast) or fp32 (exact, ~2.5x slower)
MM_BF16 = True

AF = mybir.ActivationFunctionType
ALU = mybir.AluOpType


def _build_program(mm_bf16: bool):
    """Build the per-core Bass program."""
    MD = BF if mm_bf16 else F32

    # packed weight column maps; each chunk occupies [0:K, col:col+M]
    a_cols = 64 + 64 + 4 * 128 + 4 * 128 + 64 + 4 * 128 + 4 * 128   # atc,sp,ih,hh,disp,m1h,m1a
    b_cols = 32 * 128
    c_cols = 72 * 128
    d_cols = 8 * 128
    NB = 64

    nc = bacc.Bacc(None, target_bir_lowering=False)

    wpa_d = nc.dram_tensor("wpa", [128, a_cols], MD, kind="ExternalInput")
    wpb_d = nc.dram_tensor("wpb", [128, b_cols], MD, kind="ExternalInput")
    wpc_d = nc.dram_tensor("wpc", [128, c_cols], MD, kind="ExternalInput")
    wpd_d = nc.dram_tensor("wpd", [128, d_cols], MD, kind="ExternalInput")
    bia_d = nc.dram_tensor("bia", [128, NB], F32, kind="ExternalInput")
    st_d = nc.dram_tensor("st", [128, 3 * R], MD, kind="ExternalInput")
    c0_d = nc.dram_tensor("c0t", [128, R], F32, kind="ExternalInput")

    rels_d = nc.dram_tensor("rels", [SEQ, 2, R], F32, kind="ExternalOutput")
    hout_d = nc.dram_tensor("hout", [128, R], F32, kind="ExternalOutput")

    with tile.TileContext(nc) as tc:
        with (
            tc.tile_pool(name="wgt", bufs=1) as wgt,
            tc.tile_pool(name="state", bufs=1) as state,
            tc.tile_pool(name="act", bufs=2) as act,
            tc.tile_pool(name="ps", bufs=3, space="PSUM") as ps,
            tc.tile_pool(name="psg", bufs=2, space="PSUM") as psg,
            tc.tile_pool(name="pss", bufs=1, space="PSUM") as pss,
        ):
            # ---- initial state (packed; enqueued before the big weights) ----
            stt = state.tile([128, 3 * R], MD, tag="st0")
            cT = state.tile([128, R], F32, tag="c0")
            h_bf = stt[:, 0:R]
            di_bf = stt[0:64, R:2 * R]          # host-computed lpr@W_sp + b_sp
            atT = stt[0:64, 2 * R:3 * R]        # host-computed at embedding

            def _state_dma_slot():
                nc.gpsimd.dma_start(stt[:], st_d[:])
                nc.gpsimd.dma_start(cT[:], c0_d[:])

            wpa = wgt.tile([128, a_cols], MD)
            wpb = wgt.tile([128, b_cols], MD)
            wpc = wgt.tile([128, c_cols], MD)
            wpd = wgt.tile([128, d_cols], MD)
            bia = wgt.tile([128, NB], F32)
            # order + queue spread matters: the prologue needs bia/wpa and the
            # tiny state tensors immediately; the big mid/late weights (wpb,
            # wpc, wpd) go on other queues so they don't block the start.
            # gpsimd's software-DGE queue is by far the fastest path
            # (~250 GB/s vs ~4 GB/s on the sync HWDGE queue for these 2D
            # patterns); everything startup-critical goes there, in need
            # order. wpb/wpd ride the scalar HWDGE queue in parallel.
            nc.gpsimd.dma_start(bia[:], bia_d[:])
            # placeholder: state DMAs enqueued here (before the big weights)
            _state_dma_slot()
            c0_ = 128                                  # atc+wsp
            c1_ = 128 + 4 * 128 + 4 * 128              # + ih + hh
            nc.gpsimd.dma_start(wpa[:, 0:c0_], wpa_d[:, 0:c0_])
            nc.gpsimd.dma_start(wpa[:, c0_:c1_], wpa_d[:, c0_:c1_])
            nc.gpsimd.dma_start(wpa[:, c1_:], wpa_d[:, c1_:])
            nc.scalar.dma_start(wpb[:], wpb_d[:])
            ch = 8 * 128 + 4 * 8 * 128                # d1h + first 4 k-rows of d1p
            nc.gpsimd.dma_start(wpc[:, 0:ch], wpc_d[:, 0:ch])
            nc.gpsimd.dma_start(wpc[:, ch:], wpc_d[:, ch:])
            nc.scalar.dma_start(wpd[:], wpd_d[:])

            off = [0]

            def _chunk(pool_tile, K, M):
                c = off[0]
                off[0] += M
                return pool_tile[0:K, c:c + M]

            atc = _chunk(wpa, 1, 64)
            wsp = _chunk(wpa, 2, 64)
            ih = [_chunk(wpa, 64, 128) for _ in range(4)]
            hh = [_chunk(wpa, 128, 128) for _ in range(4)]
            disp = _chunk(wpa, 128, 64)
            m1h = [_chunk(wpa, 128, 128) for _ in range(4)]
            m1a = [_chunk(wpa, 64, 128) for _ in range(4)]
            assert off[0] == a_cols
            m2w = [[wpb[:, (k * 8 + m) * 128:(k * 8 + m) * 128 + 128]
                    for m in range(8)] for k in range(4)]
            d1h = [wpc[:, m * 128:m * 128 + 128] for m in range(8)]
            d1p = [[wpc[:, (8 + k * 8 + m) * 128:(8 + k * 8 + m) * 128 + 128]
                    for m in range(8)] for k in range(8)]
            d2w = [wpd[:, k * 128:k * 128 + 128] for k in range(8)]

            bg = [bia[:, n:n + 1] for n in range(4)]          # lstm gates (i,f,g,o)
            bdisp = bia[0:64, 4:5]
            bm1 = [bia[:, 5 + n:6 + n] for n in range(4)]
            bm2 = [bia[:, 9 + m:10 + m] for m in range(8)]
            bd1 = [bia[:, 17 + m:18 + m] for m in range(8)]
            bd2 = bia[:, 25:26]
            bpat = bia[0:64, 26:27]
            bh2p = bia[0:2, 27:28]
            bsp = bia[0:64, 28:29]
            wh2p = bia[:, 29:31]
            bm2a = bia[:, 32:64]

            def gate_psums():
                # 4 gate accumulators packed into two banks: A = (i,f), B = (g,o)
                gA = psg.tile([128, 256], F32, tag="gA")
                gB = psg.tile([128, 256], F32, tag="gB")
                return [gA[:, 0:128], gA[:, 128:256], gB[:, 0:128], gB[:, 128:256]]

            def issue_ih(di_tile):
                # start=True only on the first matmul touching each bank: a
                # PSUM "start" zeroes the whole 2KB zero region (= bank).
                g = gate_psums()
                for n in range(4):
                    nc.tensor.matmul(g[n], ih[n],
                                     di_tile if isinstance(di_tile, bass.AP) else di_tile[:],
                                     start=(n % 2 == 0), stop=False)
                return g

            g_cur = issue_ih(di_bf)

            for t in range(SEQ):
                last = t == SEQ - 1
                # ---- finish gates: hh part ----
                for n in range(4):
                    nc.tensor.matmul(g_cur[n], hh[n], h_bf if isinstance(h_bf, bass.AP) else h_bf[:], start=False,
                                     stop=(n % 2 == 1))

                # ---- LSTM elementwise ----
                sig_i = act.tile([128, R], F32, tag="sig_i")
                sig_f = act.tile([128, R], F32, tag="sig_f")
                tnh_g = act.tile([128, R], F32, tag="tnh_g")
                sig_o = act.tile([128, R], F32, tag="sig_o")
                cN = state.tile([128, R], F32, tag=f"c{(t + 1) % 2}")
                tnh_c = act.tile([128, R], F32, tag="tnh_c")
                hl_bf = act.tile([128, R], MD, tag="hlbf")
                t1 = act.tile([128, R], F32, tag="t1")
                t2 = act.tile([128, R], F32, tag="t2")
                i_sigf_b = nc.scalar.activation(sig_f[:], g_cur[1], AF.Sigmoid, bias=bg[1], scale=1.0)
                nc.scalar.activation(sig_i[:], g_cur[0], AF.Sigmoid, bias=bg[0], scale=1.0)
                nc.scalar.activation(tnh_g[:], g_cur[2], AF.Tanh, bias=bg[2], scale=1.0)
                nc.scalar.activation(sig_o[:], g_cur[3], AF.Sigmoid, bias=bg[3], scale=1.0)
                i_sigf = i_sigf_b.ins
                nc.vector.tensor_mul(t1[:], sig_f[:], cT[:])
                nc.vector.tensor_mul(t2[:], sig_i[:], tnh_g[:])
                nc.vector.tensor_add(cN[:], t1[:], t2[:])
                nc.scalar.activation(tnh_c[:], cN[:], AF.Tanh, bias=0.0, scale=1.0)
                nc.vector.tensor_mul(hl_bf[:], sig_o[:], tnh_c[:])
                cT = cN

                # ---- PE heat through the LSTM-chain window: real matmuls
                # pinned (via artificial dep on sig_f) to execute inside the
                # serial elementwise stretch so HAM never re-throttles. ----
                m1_ps = []
                for n in range(4):
                    mp = ps.tile([128, R], F32, tag="ps")
                    mm = nc.tensor.matmul(mp[:], m1a[n], atT, start=True, stop=False)
                    add_dep_helper(mm.ins, i_sigf, reason="boundary-gap filler")
                    m1_ps.append(mp)
                scr = pss.tile([128, R], F32, tag="sm")
                for n in range(30):
                    mm = nc.tensor.matmul(scr[:], hh[n % 4],
                                          h_bf if isinstance(h_bf, bass.AP) else h_bf[:],
                                          start=True, stop=True)
                    add_dep_helper(mm.ins, i_sigf, reason="boundary-gap heater")
                i_hl = nc.vector.tensor_mul(hl_bf[0:1, 0:1], sig_o[0:1, 0:1],
                                            tnh_c[0:1, 0:1])

                # ---- m1 h-part ----
                for n in range(4):
                    nc.tensor.matmul(m1_ps[n][:], m1h[n], hl_bf[:],
                                     start=False, stop=True)

                # fp32 hl (only for the h2p output head, off the critical path)
                hlF = act.tile([128, R], F32, tag="hlF")
                nc.vector.tensor_mul(hlF[:], sig_o[:], tnh_c[:])
                rel_ps = pss.tile([2, R], F32, tag="sm")
                nc.tensor.matmul(rel_ps[:], wh2p, hlF[:], start=True, stop=True)
                relT = act.tile([2, R], F32, tag="rel")
                nc.scalar.activation(relT[:], rel_ps[:], AF.Identity, bias=bh2p, scale=1.0)
                nc.gpsimd.dma_start(rels_d[t], relT[:])

                if not last:
                    di_ps = pss.tile([64, R], F32, tag="sm")
                    nc.tensor.matmul(di_ps[:], disp, hl_bf[:], start=True, stop=True)
                    di_bf = act.tile([64, R], MD, tag="di")
                    nc.scalar.activation(di_bf[:], di_ps[:], AF.Identity, bias=bdisp, scale=1.0)

                # ---- m1 relu (split across ACT and DVE for latency) ----
                m1_bf = []
                for n in range(4):
                    mt = act.tile([128, R], MD, tag=f"m1s_{n}")
                    if n % 2 == 0:
                        nc.scalar.activation(mt[:], m1_ps[n][:], AF.Relu, bias=bm1[n], scale=1.0)
                    else:
                        nc.vector.tensor_scalar(mt[:], m1_ps[n][:], bm1[n], 0.0,
                                                op0=ALU.add, op1=ALU.max)
                    m1_bf.append(mt)
                for n in range(4):
                    mm = nc.tensor.matmul(scr[:], hh[n % 4],
                                          h_bf if isinstance(h_bf, bass.AP) else h_bf[:],
                                          start=True, stop=True)
                    add_dep_helper(mm.ins, i_hl.ins, reason="m1-relu-wait heater")

                # ---- m2 + fused group-max (max before bias+relu) ----
                # two output chunks share one PSUM bank -> one zero-region
                # start per pair instead of per chunk
                ph = []
                for mp_i in range(4):
                    mp = ps.tile([128, 2 * R], F32, tag="ps")
                    for half in range(2):
                        m = 2 * mp_i + half
                        sl = mp[:, half * R:(half + 1) * R]
                        for k in range(4):
                            nc.tensor.matmul(sl, m2w[k][m], m1_bf[k][:],
                                             start=(half == 0 and k == 0),
                                             stop=(half == 1 and k == 3))
                    for half in range(2):
                        m = 2 * mp_i + half
                        sl = mp[:, half * R:(half + 1) * R]
                        pr = act.tile([128, GC], F32, tag=f"phr_{m}")
                        nc.vector.reduce_max(pr[:], sl.rearrange("p (g j) -> p g j", j=P),
                                             axis=mybir.AxisListType.X)
                        pt = act.tile([128, GC], MD, tag=f"ph_{m}")
                        nc.vector.tensor_scalar(pt[:], pr[:], bm2[m], 0.0,
                                                op0=ALU.add, op1=ALU.max)
                        ph.append(pt)

                # ---- d1 (rotated ph order; two chunks per PSUM bank) ----
                d1_bf = [None] * 8
                for dp_i in range(4):
                    dp = ps.tile([128, 2 * R], F32, tag="ps")
                    for half in range(2):
                        m = 2 * dp_i + half
                        sl = dp[:, half * R:(half + 1) * R]
                        nc.tensor.matmul(sl, d1h[m], hl_bf[:],
                                         start=(half == 0), stop=False)
                        for j in range(8):
                            k = (m + j) % 8
                            nc.tensor.matmul(sl, d1p[k][m],
                                             ph[k][:].broadcast_to((128, GC, P)),
                                             start=False,
                                             stop=(half == 1 and j == 7))
                    for half in range(2):
                        m = 2 * dp_i + half
                        sl = dp[:, half * R:(half + 1) * R]
                        dt_ = act.tile([128, R], MD, tag=f"d1s_{m % 4}")
                        if half == 0:
                            nc.scalar.activation(dt_[:], sl, AF.Relu, bias=bd1[m], scale=1.0)
                        else:
                            nc.vector.tensor_scalar(dt_[:], sl, bd1[m], 0.0,
                                                    op0=ALU.add, op1=ALU.max)
                        d1_bf[m] = dt_

                # ---- d2 (accumulate in d1 completion order: 1,2,...,7,0) ----
                d2_ps = ps.tile([128, R], F32, tag="ps")
                order = [0, 1, 2, 3, 4, 5, 6, 7]
                for j, m in enumerate(order):
                    nc.tensor.matmul(d2_ps[:], d2w[m], d1_bf[m][:],
                                     start=(j == 0), stop=(j == 7))
                if not last:
                    h_bf = state.tile([128, R], MD, tag=f"hcarry{(t + 1) % 2}")
                    # DVE is idle here and ~100ns faster than ACT for this
                    nc.vector.tensor_scalar(h_bf[:], d2_ps[:], bd2, 0.0,
                                            op0=ALU.add, op1=ALU.max)
                    # issue next step's input-gate matmuls late so the
                    # scheduler uses them as PE filler during the boundary
                    g_cur = issue_ih(di_bf)
                else:
                    h_f32 = state.tile([128, R], F32, tag="hfin")
                    nc.scalar.activation(h_f32[:], d2_ps[:], AF.Relu, bias=bd2, scale=1.0)
                    nc.gpsimd.dma_start(hout_d[:], h_f32[:])

    nc.compile()
    return nc, None


def _pack_weights(ins, mm_bf16: bool):
    """Host-side weight packing into the per-core packed arrays."""
    MDnp = bf16 if mm_bf16 else np.float32
    f32 = np.float32

    W_ih = np.asarray(ins["W_ih"], f32)
    W_hh = np.asarray(ins["W_hh"], f32)
    W_sp = np.asarray(ins["W_sp"], f32)
    W_h2p = np.asarray(ins["W_h2p"], f32)
    Wm1 = np.asarray(ins["Wm1"], f32) * (0.05 * BN_S)
    Wm2 = np.asarray(ins["Wm2"], f32) * BN_S
    Wd1 = np.asarray(ins["Wd1"], f32) * BN_S
    Wd2 = np.asarray(ins["Wd2"], f32) * BN_S
    Wp_at = np.asarray(ins["Wp_at"], f32)

    a_chunks = [(1, Wp_at.sum(0, keepdims=True)),             # atc (1,64)
                (2, W_sp)]                                    # wsp (2,64)
    for n in range(4):
        a_chunks.append((64, W_ih[:, n * 128:(n + 1) * 128]))
    for n in range(4):
        a_chunks.append((128, W_hh[:, n * 128:(n + 1) * 128]))
    a_chunks.append((128, W_h2p @ W_sp))                      # disp (128,64)
    for n in range(4):
        a_chunks.append((128, Wm1[0:128, n * 128:(n + 1) * 128]))
    for n in range(4):
        a_chunks.append((64, Wm1[128:192, n * 128:(n + 1) * 128]))

    def pack(chunks):
        ncols = sum(c.shape[1] for _, c in chunks)
        out = np.zeros((128, ncols), dtype=MDnp)
        col = 0
        for K, c in chunks:
            assert c.shape[0] == K
            out[0:K, col:col + c.shape[1]] = c.astype(MDnp)
            col += c.shape[1]
        return out

    wpa = pack(a_chunks)
    wpb = pack([(128, Wm2[k * 128:(k + 1) * 128, m * 128:(m + 1) * 128])
                for k in range(4) for m in range(8)])
    wpc = pack([(128, Wd1[0:128, m * 128:(m + 1) * 128]) for m in range(8)] +
               [(128, Wd1[128 + k * 128:128 + (k + 1) * 128, m * 128:(m + 1) * 128])
                for k in range(8) for m in range(8)])
    wpd = pack([(128, Wd2[k * 128:(k + 1) * 128, 0:128]) for k in range(8)])

    b_ih = np.asarray(ins["b_ih"], f32)
    b_hh = np.asarray(ins["b_hh"], f32)
    b_sp = np.asarray(ins["b_sp"], f32)
    b_h2p = np.asarray(ins["b_h2p"], f32)
    bm1 = np.asarray(ins["bm1"], f32) * BN_S
    bm2 = np.asarray(ins["bm2"], f32) * BN_S
    bd1 = np.asarray(ins["bd1"], f32) * BN_S
    bd2 = np.asarray(ins["bd2"], f32) * BN_S
    bp_at = np.asarray(ins["bp_at"], f32)
    bg = b_ih + b_hh

    bia = np.zeros((128, 64), dtype=f32)
    for n in range(4):
        bia[:, n] = bg[n * 128:(n + 1) * 128]
    bia[0:64, 4] = b_h2p @ W_sp + b_sp                        # bdisp
    for n in range(4):
        bia[:, 5 + n] = bm1[n * 128:(n + 1) * 128]
    for m in range(8):
        bia[:, 9 + m] = bm2[m * 128:(m + 1) * 128]
    for m in range(8):
        bia[:, 17 + m] = bd1[m * 128:(m + 1) * 128]
    bia[:, 25] = bd2
    bia[0:64, 26] = bp_at
    bia[0:2, 27] = b_h2p
    bia[0:64, 28] = b_sp
    bia[:, 29:31] = W_h2p
    for m in range(8):
        for g in range(4):
            bia[:, 32 + m * 4 + g] = bm2[m * 128:(m + 1) * 128]

    return dict(wpa=wpa, wpb=wpb, wpc=wpc, wpd=wpd, bia=bia)


_CACHE = {}


def _get_program(mm_bf16: bool):
    if mm_bf16 not in _CACHE:
        _CACHE[mm_bf16] = _build_program(mm_bf16)
    return _CACHE[mm_bf16]


def run(inputs, mm_bf16=MM_BF16, trace=False, **spmd_kwargs):
    ins = {k: np.asarray(v) for k, v in inputs.items()}
    nc, _ = _get_program(mm_bf16)
    wmap = _pack_weights(ins, mm_bf16)

    MDnp = bf16 if mm_bf16 else np.float32
    h0 = ins["h0"][0].astype(np.float32)        # (B,HD)
    c0 = ins["c0"][0].astype(np.float32)
    at0 = ins["agent_type"][0, :, 0].astype(np.float32)   # (B,)
    lpr = ins["last_pos_rel"].astype(np.float32)          # (B,2)
    W_sp = np.asarray(ins["W_sp"], np.float32)
    b_sp = np.asarray(ins["b_sp"], np.float32)
    Wp_at_sum = np.asarray(ins["Wp_at"], np.float32).sum(0)
    bp_at = np.asarray(ins["bp_at"], np.float32)

    in_maps = []
    for k in range(NCORES):
        rows = slice(k * R, (k + 1) * R)
        m = dict(wmap)
        st = np.zeros((128, 3 * R), dtype=MDnp)
        st[:, 0:R] = h0[rows].T.astype(MDnp)
        di0 = lpr[rows] @ W_sp + b_sp                     # (R, 64) fp32
        st[0:64, R:2 * R] = di0.T.astype(MDnp)
        at_emb = at0[rows, None] * Wp_at_sum[None, :] + bp_at[None, :]
        st[0:64, 2 * R:3 * R] = at_emb.T.astype(MDnp)
        m["st"] = st
        m["c0t"] = np.ascontiguousarray(c0[rows].T)
        in_maps.append(m)

    res = run_bass_kernel_spmd(nc, in_maps, list(range(NCORES)),
                               trace=trace, **spmd_kwargs)

    rels = np.empty((SEQ, B, 2), dtype=np.float32)
    hout = np.empty((1, B, HD), dtype=np.float32)
    for k in range(NCORES):
        rows = slice(k * R, (k + 1) * R)
        rk = res.results[k]["rels"]            # (SEQ,2,R)
        rels[:, rows, :] = np.transpose(rk, (0, 2, 1))
        hout[0, rows, :] = res.results[k]["hout"].T
    return (rels, hout), res


def kernel(**inputs):
    (rels, hout), _ = run(inputs)
    return rels, hout


# revision 31
# speedup vs baseline: 1.0599x; 1.0029x over previous
"""Trainium2 Bass kernel for nn_Decoder_20701742366850.

Math notes (verified against the reference to fp32 precision):
  * softmax over a size-1 axis is identically 1.0, so the attention-score
    branch (Wa1/Wa2 and the pairwise rel embedding) never affects the output.
  * The pooled tensor wt[s,i,j,:] = 0.05*base[s,j,:] is independent of i, so
    the P x P pooling collapses to a per-row MLP followed by a max over each
    group, broadcast back to the group's rows.
  * Positions (last_pos/cur) and velocity embeddings are dead downstream.
  * max_j relu(x_j + b) = relu(max_j x_j + b): the group max is taken on the
    raw matmul accumulator and bias+relu applied to the (128, G) result.

What remains per step, per row: LSTM cell -> h2p -> (m1 -> m2 -> group max)
-> d1 -> d2, which is what the device kernel computes.

Sharding: data parallel over the 32 scenes; each of the 8 cores owns 4
contiguous groups = 128 rows. All weights replicated; no cross-core comms.
Layout on device is feature-major (features on partitions, rows on the free
axis), so per-feature biases are per-partition ACT biases and the group max
is a free-axis windowed reduction. The group-max broadcast back to rows is
done with a stride-0 rhs access pattern on the d1 matmuls (free).
"""

import numpy as np

try:
    import concourse.bass as bass  # noqa: F401
except Exception:  # pragma: no cover - fresh grading dir
    import sys
    for _p in ("/opt/trn_rl_repo",):
        if _p not in sys.path:
            sys.path.insert(0, _p)

import concourse.bacc as bacc
import concourse.bass as bass
import concourse.mybir as mybir
from concourse import tile
from concourse.tile_rust import add_dep_helper
from concourse.bass_utils import run_bass_kernel_spmd
import ml_dtypes

bf16 = ml_dtypes.bfloat16

SEQ = 12
E = 64
HD = 128
G = 32
P = 32
B = G * P
BOT = 1024
MLP = 1024
NCORES = 8
R = B // NCORES            # rows per core
GC = G // NCORES           # groups per core
H = R // 2                 # free-dim half for latency-split elementwise ops
BN_S = float(1.0 / np.sqrt(1.0 + 1e-5))

F32 = mybir.dt.float32
BF = mybir.dt.bfloat16

# matmul stream dtype: bf16 (fast) or fp32 (exact, ~2.5x slower)
MM_BF16 = True

AF = mybir.ActivationFunctionType
ALU = mybir.AluOpType


def _build_program(mm_bf16: bool):
    """Build the per-core Bass program."""
    MD = BF if mm_bf16 else F32

    # packed weight column maps; each chunk occupies [0:K, col:col+M]
    a_cols = 64 + 64 + 4 * 128 + 4 * 128 + 64 + 4 * 128 + 4 * 128   # atc,sp,ih,hh,disp,m1h,m1a
    b_cols = 32 * 128
    c_cols = 72 * 128
    d_cols = 8 * 128
    NB = 64

    nc = bacc.Bacc(None, target_bir_lowering=False)

    wpa_d = nc.dram_tensor("wpa", [128, a_cols], MD, kind="ExternalInput")
    wpb_d = nc.dram_tensor("wpb", [128, b_cols], MD, kind="ExternalInput")
    wpc_d = nc.dram_tensor("wpc", [128, c_cols], MD, kind="ExternalInput")
    wpd_d = nc.dram_tensor("wpd", [128, d_cols], MD, kind="ExternalInput")
    bia_d = nc.dram_tensor("bia", [128, NB], F32, kind="ExternalInput")
    st_d = nc.dram_tensor("st", [128, 3 * R], MD, kind="ExternalInput")
    c0_d = nc.dram_tensor("c0t", [128, R], F32, kind="ExternalInput")

    rels_d = nc.dram_tensor("rels", [SEQ, 2, R], F32, kind="ExternalOutput")
    hout_d = nc.dram_tensor("hout", [128, R], F32, kind="ExternalOutput")

    with tile.TileContext(nc) as tc:
        with (
            tc.tile_pool(name="wgt", bufs=1) as wgt,
            tc.tile_pool(name="state", bufs=1) as state,
            tc.tile_pool(name="act", bufs=2) as act,
            tc.tile_pool(name="ps", bufs=3, space="PSUM") as ps,
            tc.tile_pool(name="psg", bufs=2, space="PSUM") as psg,
            tc.tile_pool(name="pss", bufs=1, space="PSUM") as pss,
        ):
            # ---- initial state (packed; enqueued before the big weights) ----
            stt = state.tile([128, 3 * R], MD, tag="st0")
            cT = state.tile([128, R], F32, tag="c0")
            h_bf = stt[:, 0:R]
            di_bf = stt[0:64, R:2 * R]          # host-computed lpr@W_sp + b_sp
            atT = stt[0:64, 2 * R:3 * R]        # host-computed at embedding

            def _state_dma_slot():
                nc.gpsimd.dma_start(stt[:], st_d[:])
                nc.gpsimd.dma_start(cT[:], c0_d[:])

            wpa = wgt.tile([128, a_cols], MD)
            wpb = wgt.tile([128, b_cols], MD)
            wpc = wgt.tile([128, c_cols], MD)
            wpd = wgt.tile([128, d_cols], MD)
            bia = wgt.tile([128, NB], F32)
            # order + queue spread matters: the prologue needs bia/wpa and the
            # tiny state tensors immediately; the big mid/late weights (wpb,
            # wpc, wpd) go on other queues so they don't block the start.
            # gpsimd's software-DGE queue is by far the fastest path
            # (~250 GB/s vs ~4 GB/s on the sync HWDGE queue for these 2D
            # patterns); everything startup-critical goes there, in need
            # order. wpb/wpd ride the scalar HWDGE queue in parallel.
            nc.gpsimd.dma_start(bia[:], bia_d[:])
            # placeholder: state DMAs enqueued here (before the big weights)
            _state_dma_slot()
            c0_ = 128                                  # atc+wsp
            c1_ = 128 + 4 * 128 + 4 * 128              # + ih + hh
            nc.gpsimd.dma_start(wpa[:, 0:c0_], wpa_d[:, 0:c0_])
            nc.gpsimd.dma_start(wpa[:, c0_:c1_], wpa_d[:, c0_:c1_])
            nc.gpsimd.dma_start(wpa[:, c1_:], wpa_d[:, c1_:])
            nc.scalar.dma_start(wpb[:], wpb_d[:])
            ch = 8 * 128 + 4 * 8 * 128                # d1h + first 4 k-rows of d1p
            nc.gpsimd.dma_start(wpc[:, 0:ch], wpc_d[:, 0:ch])
            nc.gpsimd.dma_start(wpc[:, ch:], wpc_d[:, ch:])
            nc.scalar.dma_start(wpd[:], wpd_d[:])

            off = [0]

            def _chunk(pool_tile, K, M):
                c = off[0]
                off[0] += M
                return pool_tile[0:K, c:c + M]

            atc = _chunk(wpa, 1, 64)
            wsp = _chunk(wpa, 2, 64)
            ih = [_chunk(wpa, 64, 128) for _ in range(4)]
            hh = [_chunk(wpa, 128, 128) for _ in range(4)]
            disp = _chunk(wpa, 128, 64)
            m1h = [_chunk(wpa, 128, 128) for _ in range(4)]
            m1a = [_chunk(wpa, 64, 128) for _ in range(4)]
            assert off[0] == a_cols
            m2w = [[wpb[:, (k * 8 + m) * 128:(k * 8 + m) * 128 + 128]
                    for m in range(8)] for k in range(4)]
            d1h = [wpc[:, m * 128:m * 128 + 128] for m in range(8)]
            d1p = [[wpc[:, (8 + k * 8 + m) * 128:(8 + k * 8 + m) * 128 + 128]
                    for m in range(8)] for k in range(8)]
            d2w = [wpd[:, k * 128:k * 128 + 128] for k in range(8)]

            bg = [bia[:, n:n + 1] for n in range(4)]          # lstm gates (i,f,g,o)
            bdisp = bia[0:64, 4:5]
            bm1 = [bia[:, 5 + n:6 + n] for n in range(4)]
            bm2 = [bia[:, 9 + m:10 + m] for m in range(8)]
            bd1 = [bia[:, 17 + m:18 + m] for m in range(8)]
            bd2 = bia[:, 25:26]
            bpat = bia[0:64, 26:27]
            bh2p = bia[0:2, 27:28]
            bsp = bia[0:64, 28:29]
            wh2p = bia[:, 29:31]
            bm2a = bia[:, 32:64]

            def gate_psums():
                # 4 gate accumulators packed into two banks: A = (i,f), B = (g,o)
                gA = psg.tile([128, 256], F32, tag="gA")
                gB = psg.tile([128, 256], F32, tag="gB")
                return [gA[:, 0:128], gA[:, 128:256], gB[:, 0:128], gB[:, 128:256]]

            def issue_ih(di_tile):
                # start=True only on the first matmul touching each bank: a
                # PSUM "start" zeroes the whole 2KB zero region (= bank).
                g = gate_psums()
                for n in range(4):
                    nc.tensor.matmul(g[n], ih[n],
                                     di_tile if isinstance(di_tile, bass.AP) else di_tile[:],
                                     start=(n % 2 == 0), stop=False)
                return g

            g_cur = issue_ih(di_bf)
            i_hrel = None

            for t in range(SEQ):
                last = t == SEQ - 1
                # ---- finish gates: hh part ----
                for n in range(4):
                    nc.tensor.matmul(g_cur[n], hh[n], h_bf if isinstance(h_bf, bass.AP) else h_bf[:], start=False,
                                     stop=(n % 2 == 1))

                # ---- LSTM elementwise ----
                sig_i = act.tile([128, R], F32, tag="sig_i")
                sig_f = act.tile([128, R], F32, tag="sig_f")
                tnh_g = act.tile([128, R], F32, tag="tnh_g")
                sig_o = act.tile([128, R], F32, tag="sig_o")
                cN = state.tile([128, R], F32, tag=f"c{(t + 1) % 2}")
                tnh_c = act.tile([128, R], F32, tag="tnh_c")
                hl_bf = act.tile([128, R], MD, tag="hlbf")
                t1 = act.tile([128, R], F32, tag="t1")
                t2 = act.tile([128, R], F32, tag="t2")
                i_sigf_b = nc.scalar.activation(sig_f[:], g_cur[1], AF.Sigmoid, bias=bg[1], scale=1.0)
                nc.scalar.activation(sig_i[:], g_cur[0], AF.Sigmoid, bias=bg[0], scale=1.0)
                nc.scalar.activation(tnh_g[:], g_cur[2], AF.Tanh, bias=bg[2], scale=1.0)
                nc.scalar.activation(sig_o[:], g_cur[3], AF.Sigmoid, bias=bg[3], scale=1.0)
                i_sigf = i_sigf_b.ins
                nc.vector.tensor_mul(t1[:], sig_f[:], cT[:])
                nc.vector.tensor_mul(t2[:], sig_i[:], tnh_g[:])
                nc.vector.tensor_add(cN[:], t1[:], t2[:])
                nc.scalar.activation(tnh_c[:], cN[:], AF.Tanh, bias=0.0, scale=1.0)
                nc.vector.tensor_mul(hl_bf[:], sig_o[:], tnh_c[:])
                cT = cN

                # ---- PE heat through the LSTM-chain window: real matmuls
                # pinned (via artificial dep on sig_f) to execute inside the
                # serial elementwise stretch so HAM never re-throttles. ----
                m1_ps = []
                for n in range(4):
                    mp = ps.tile([128, R], F32, tag="ps")
                    mm = nc.tensor.matmul(mp[:], m1a[n], atT, start=True, stop=False)
                    add_dep_helper(mm.ins, i_sigf, reason="boundary-gap filler")
                    m1_ps.append(mp)
                scr = pss.tile([128, R], F32, tag="sm")
                for n in range(34):
                    mm = nc.tensor.matmul(scr[:], hh[n % 4],
                                          h_bf if isinstance(h_bf, bass.AP) else h_bf[:],
                                          start=True, stop=True)
                    add_dep_helper(mm.ins, i_hrel if i_hrel is not None else i_sigf,
                                   reason="boundary-gap heater")
                i_hl = nc.vector.tensor_mul(hl_bf[0:1, 0:1], sig_o[0:1, 0:1],
                                            tnh_c[0:1, 0:1])

                # ---- m1 h-part ----
                for n in range(4):
                    nc.tensor.matmul(m1_ps[n][:], m1h[n], hl_bf[:],
                                     start=False, stop=True)

                # fp32 hl (only for the h2p output head, off the critical path)
                hlF = act.tile([128, R], F32, tag="hlF")
                nc.vector.tensor_mul(hlF[:], sig_o[:], tnh_c[:])
                rel_ps = pss.tile([2, R], F32, tag="sm")
                nc.tensor.matmul(rel_ps[:], wh2p, hlF[:], start=True, stop=True)
                relT = act.tile([2, R], F32, tag="rel")
                nc.scalar.activation(relT[:], rel_ps[:], AF.Identity, bias=bh2p, scale=1.0)
                nc.gpsimd.dma_start(rels_d[t], relT[:])

                if not last:
                    di_ps = pss.tile([64, R], F32, tag="sm")
                    nc.tensor.matmul(di_ps[:], disp, hl_bf[:], start=True, stop=True)
                    di_bf = act.tile([64, R], MD, tag="di")
                    nc.scalar.activation(di_bf[:], di_ps[:], AF.Identity, bias=bdisp, scale=1.0)

                # ---- m1 relu (split across ACT and DVE for latency) ----
                m1_bf = []
                for n in range(4):
                    mt = act.tile([128, R], MD, tag=f"m1s_{n}")
                    if n % 2 == 0:
                        nc.scalar.activation(mt[:], m1_ps[n][:], AF.Relu, bias=bm1[n], scale=1.0)
                    else:
                        nc.vector.tensor_scalar(mt[:], m1_ps[n][:], bm1[n], 0.0,
                                                op0=ALU.add, op1=ALU.max)
                    m1_bf.append(mt)
                for n in range(8):
                    mm = nc.tensor.matmul(scr[:], hh[n % 4],
                                          h_bf if isinstance(h_bf, bass.AP) else h_bf[:],
                                          start=True, stop=True)
                    add_dep_helper(mm.ins, i_hl.ins, reason="m1-relu-wait heater")

                # ---- m2 + fused group-max (max before bias+relu) ----
                # two output chunks share one PSUM bank -> one zero-region
                # start per pair instead of per chunk
                ph = []
                for mp_i in range(4):
                    mp = ps.tile([128, 2 * R], F32, tag="ps")
                    for half in range(2):
                        m = 2 * mp_i + half
                        sl = mp[:, half * R:(half + 1) * R]
                        for k in range(4):
                            nc.tensor.matmul(sl, m2w[k][m], m1_bf[k][:],
                                             start=(half == 0 and k == 0),
                                             stop=(half == 1 and k == 3))
                    for half in range(2):
                        m = 2 * mp_i + half
                        sl = mp[:, half * R:(half + 1) * R]
                        pr = act.tile([128, GC], F32, tag=f"phr_{m}")
                        nc.vector.reduce_max(pr[:], sl.rearrange("p (g j) -> p g j", j=P),
                                             axis=mybir.AxisListType.X)
                        pt = act.tile([128, GC], MD, tag=f"ph_{m}")
                        nc.vector.tensor_scalar(pt[:], pr[:], bm2[m], 0.0,
                                                op0=ALU.add, op1=ALU.max)
                        ph.append(pt)

                # ---- d1 (rotated ph order; two chunks per PSUM bank) ----
                d1_bf = [None] * 8
                for dp_i in range(4):
                    dp = ps.tile([128, 2 * R], F32, tag="ps")
                    for half in range(2):
                        m = 2 * dp_i + half
                        sl = dp[:, half * R:(half + 1) * R]
                        nc.tensor.matmul(sl, d1h[m], hl_bf[:],
                                         start=(half == 0), stop=False)
                        for j in range(8):
                            k = (m + j) % 8
                            nc.tensor.matmul(sl, d1p[k][m],
                                             ph[k][:].broadcast_to((128, GC, P)),
                                             start=False,
                                             stop=(half == 1 and j == 7))
                    for half in range(2):
                        m = 2 * dp_i + half
                        sl = dp[:, half * R:(half + 1) * R]
                        dt_ = act.tile([128, R], MD, tag=f"d1s_{m % 4}")
                        if half == 0:
                            nc.scalar.activation(dt_[:], sl, AF.Relu, bias=bd1[m], scale=1.0)
                        else:
                            nc.vector.tensor_scalar(dt_[:], sl, bd1[m], 0.0,
                                                    op0=ALU.add, op1=ALU.max)
                        d1_bf[m] = dt_

                # ---- d2 (accumulate in d1 completion order: 1,2,...,7,0) ----
                d2_ps = ps.tile([128, R], F32, tag="ps")
                order = [0, 1, 2, 3, 4, 5, 6, 7]
                for j, m in enumerate(order):
                    nc.tensor.matmul(d2_ps[:], d2w[m], d1_bf[m][:],
                                     start=(j == 0), stop=(j == 7))
                if not last:
                    h_bf = state.tile([128, R], MD, tag=f"hcarry{(t + 1) % 2}")
                    # DVE is idle here and ~100ns faster than ACT for this
                    i_hrel = nc.vector.tensor_scalar(h_bf[:], d2_ps[:], bd2, 0.0,
                                                     op0=ALU.add, op1=ALU.max).ins
                    # issue next step's input-gate matmuls late so the
                    # scheduler uses them as PE filler during the boundary
                    g_cur = issue_ih(di_bf)
                else:
                    h_f32 = state.tile([128, R], F32, tag="hfin")
                    nc.scalar.activation(h_f32[:], d2_ps[:], AF.Relu, bias=bd2, scale=1.0)
                    nc.gpsimd.dma_start(hout_d[:], h_f32[:])

    nc.compile()
    return nc, None


def _pack_weights(ins, mm_bf16: bool):
    """Host-side weight packing into the per-core packed arrays."""
    MDnp = bf16 if mm_bf16 else np.float32
    f32 = np.float32

    W_ih = np.asarray(ins["W_ih"], f32)
    W_hh = np.asarray(ins["W_hh"], f32)
    W_sp = np.asarray(ins["W_sp"], f32)
    W_h2p = np.asarray(ins["W_h2p"], f32)
    Wm1 = np.asarray(ins["Wm1"], f32) * (0.05 * BN_S)
    Wm2 = np.asarray(ins["Wm2"], f32) * BN_S
    Wd1 = np.asarray(ins["Wd1"], f32) * BN_S
    Wd2 = np.asarray(ins["Wd2"], f32) * BN_S
    Wp_at = np.asarray(ins["Wp_at"], f32)

    a_chunks = [(1, Wp_at.sum(0, keepdims=True)),             # atc (1,64)
                (2, W_sp)]                                    # wsp (2,64)
    for n in range(4):
        a_chunks.append((64, W_ih[:, n * 128:(n + 1) * 128]))
    for n in range(4):
        a_chunks.append((128, W_hh[:, n * 128:(n + 1) * 128]))
    a_chunks.append((128, W_h2p @ W_sp))                      # disp (128,64)
    for n in range(4):
        a_chunks.append((128, Wm1[0:128, n * 128:(n + 1) * 128]))
    for n in range(4):
        a_chunks.append((64, Wm1[128:192, n * 128:(n + 1) * 128]))

    def pack(chunks):
        ncols = sum(c.shape[1] for _, c in chunks)
        out = np.zeros((128, ncols), dtype=MDnp)
        col = 0
        for K, c in chunks:
            assert c.shape[0] == K
            out[0:K, col:col + c.shape[1]] = c.astype(MDnp)
            col += c.shape[1]
        return out

    wpa = pack(a_chunks)
    wpb = pack([(128, Wm2[k * 128:(k + 1) * 128, m * 128:(m + 1) * 128])
                for k in range(4) for m in range(8)])
    wpc = pack([(128, Wd1[0:128, m * 128:(m + 1) * 128]) for m in range(8)] +
               [(128, Wd1[128 + k * 128:128 + (k + 1) * 128, m * 128:(m + 1) * 128])
                for k in range(8) for m in range(8)])
    wpd = pack([(128, Wd2[k * 128:(k + 1) * 128, 0:128]) for k in range(8)])

    b_ih = np.asarray(ins["b_ih"], f32)
    b_hh = np.asarray(ins["b_hh"], f32)
    b_sp = np.asarray(ins["b_sp"], f32)
    b_h2p = np.asarray(ins["b_h2p"], f32)
    bm1 = np.asarray(ins["bm1"], f32) * BN_S
    bm2 = np.asarray(ins["bm2"], f32) * BN_S
    bd1 = np.asarray(ins["bd1"], f32) * BN_S
    bd2 = np.asarray(ins["bd2"], f32) * BN_S
    bp_at = np.asarray(ins["bp_at"], f32)
    bg = b_ih + b_hh

    bia = np.zeros((128, 64), dtype=f32)
    for n in range(4):
        bia[:, n] = bg[n * 128:(n + 1) * 128]
    bia[0:64, 4] = b_h2p @ W_sp + b_sp                        # bdisp
    for n in range(4):
        bia[:, 5 + n] = bm1[n * 128:(n + 1) * 128]
    for m in range(8):
        bia[:, 9 + m] = bm2[m * 128:(m + 1) * 128]
    for m in range(8):
        bia[:, 17 + m] = bd1[m * 128:(m + 1) * 128]
    bia[:, 25] = bd2
    bia[0:64, 26] = bp_at
    bia[0:2, 27] = b_h2p
    bia[0:64, 28] = b_sp
    bia[:, 29:31] = W_h2p
    for m in range(8):
        for g in range(4):
            bia[:, 32 + m * 4 + g] = bm2[m * 128:(m + 1) * 128]

    return dict(wpa=wpa, wpb=wpb, wpc=wpc, wpd=wpd, bia=bia)


_CACHE = {}


def _get_program(mm_bf16: bool):
    if mm_bf16 not in _CACHE:
        _CACHE[mm_bf16] = _build_program(mm_bf16)
    return _CACHE[mm_bf16]


def run(inputs, mm_bf16=MM_BF16, trace=False, **spmd_kwargs):
    ins = {k: np.asarray(v) for k, v in inputs.items()}
    nc, _ = _get_program(mm_bf16)
    wmap = _pack_weights(ins, mm_bf16)

    MDnp = bf16 if mm_bf16 else np.float32
    h0 = ins["h0"][0].astype(np.float32)        # (B,HD)
    c0 = ins["c0"][0].astype(np.float32)
    at0 = ins["agent_type"][0, :, 0].astype(np.float32)   # (B,)
    lpr = ins["last_pos_rel"].astype(np.float32)          # (B,2)
    W_sp = np.asarray(ins["W_sp"], np.float32)
    b_sp = np.asarray(ins["b_sp"], np.float32)
    Wp_at_sum = np.asarray(ins["Wp_at"], np.float32).sum(0)
    bp_at = np.asarray(ins["bp_at"], np.float32)

    in_maps = []
    for k in range(NCORES):
        rows = slice(k * R, (k + 1) * R)
        m = dict(wmap)
        st = np.zeros((128, 3 * R), dtype=MDnp)
        st[:, 0:R] = h0[rows].T.astype(MDnp)
        di0 = lpr[rows] @ W_sp + b_sp                     # (R, 64) fp32
        st[0:64, R:2 * R] = di0.T.astype(MDnp)
        at_emb = at0[rows, None] * Wp_at_sum[None, :] + bp_at[None, :]
        st[0:64, 2 * R:3 * R] = at_emb.T.astype(MDnp)
        m["st"] = st
        m["c0t"] = np.ascontiguousarray(c0[rows].T)
        in_maps.append(m)

    res = run_bass_kernel_spmd(nc, in_maps, list(range(NCORES)),
                               trace=trace, **spmd_kwargs)

    rels = np.empty((SEQ, B, 2), dtype=np.float32)
    hout = np.empty((1, B, HD), dtype=np.float32)
    for k in range(NCORES):
        rows = slice(k * R, (k + 1) * R)
        rk = res.results[k]["rels"]            # (SEQ,2,R)
        rels[:, rows, :] = np.transpose(rk, (0, 2, 1))
        hout[0, rows, :] = res.results[k]["hout"].T
    return (rels, hout), res


def kernel(**inputs):
    (rels, hout), _ = run(inputs)
    return rels, hout
